# revision 1
# baseline (speedup 1.0000x reference)
"""Causal self-attention (dense transformer) on 8 Trainium2 NeuronCores.

Problem: x[2, 2048, 1024], W_qkv[1024, 3072], b_qkv[3072], W_out[1024, 1024],
b_out[1024]; 16 heads, head_dim 64, causal softmax attention.

Sharding: 8 cores = 2 (batch) x 4 (head groups of 4 heads). Each core computes
QKV projection for its 4 heads, full causal attention for them, and a partial
output projection (its heads' rows of W_out). Host sums the 4 partials per
batch and adds the (bias) terms.

Device-side math notes:
  - K bias is dropped: adding a constant vector to every key shifts each
    query's scores by a per-query constant -> softmax invariant.
  - V bias is folded into the output bias on host: probs row-sums are 1, so
    attn = P @ (V + 1 c^T) = P@V + 1 c^T, and c^T @ W_out is a constant row.
  - Softmax has no max-subtraction: scores/8 have |.| < ~10 here, exp is safe.
  - Scores are computed transposed (S^T[k, q]) so no transposes are needed
    anywhere: softmax denominators come from a ones-column appended to V,
    and attention output lands directly in the [head_dim, token] layout the
    output projection needs as lhsT.
  - Strictly-above-diagonal k-tiles are never computed; the 128x128 blocks on
    the diagonal get a triangular -30000 additive mask before exp, and the
    left-of-diagonal garbage columns inside a diagonal k-tile are simply
    never read by the PV matmul (its rhs is sliced to the valid q range).
  - The two heads of a pair sit on partitions 0-63 / 64-127, so their S^T
    matmuls land on disjoint PE row-groups and run concurrently.
  - Precision: score path (x, W_qkv, Q, K -> scores) in bf16; value path
    (V, probs, attention out, W_out) in float32r (TF32-like). Softmax damps
    the score-path rounding; the value path touches the output directly.
"""

import math
import os

import ml_dtypes
import numpy as np

import concourse.bass as bass
import concourse.tile as tile
from concourse import bacc, mybir
from concourse.bass_utils import run_bass_kernel_spmd

B = 2
L = 2048
D = 1024
H = 16
HD = 64
NCORES = 8
GROUPS = 4  # head groups (tensor parallel)
HPG = H // GROUPS  # heads per group = 4
DG = HPG * HD  # 256 output dims per group
KC = D // 128  # 8 contraction chunks for QKV
LT = L // 128  # 16 token tiles
QC = L // 512  # 4 query chunks of 512
MASK_VAL = -30000.0

f32 = mybir.dt.float32


def _dt(name):
    if name == "bf16":
        return mybir.dt.bfloat16, np.dtype(ml_dtypes.bfloat16)
    if name == "f32r":
        return mybir.dt.float32r, np.dtype(np.float32)
    raise ValueError(name)


# score path: x, W_qkv (q|k and v halves share x), Q^T/K^T tiles
in_dt, np_in_dt = _dt(os.environ.get("ATTN_IN_DT", "f32r"))
# value path: V tiles + exp(P^T) tiles (PV matmul operands)
val_dt, np_val_dt = _dt(os.environ.get("ATTN_VAL_DT", "f32r"))
# projection path: attention-out tiles + W_out
pj_dt, np_pj_dt = _dt(os.environ.get("ATTN_PJ_DT", "f32r"))

_CACHE = {}


def _build():
    nc = bacc.Bacc("TRN2", target_bir_lowering=False, debug=False,
                   num_devices=NCORES)

    xT = nc.dram_tensor("xT", [KC, 128, L], in_dt, kind="ExternalInput").ap()
    wqk = nc.dram_tensor("wqk", [KC, 128, 2 * DG], in_dt,
                         kind="ExternalInput").ap()
    wv = nc.dram_tensor("wv", [KC, 128, DG], in_dt, kind="ExternalInput").ap()
    wout = nc.dram_tensor("wout", [2, 128, D], pj_dt,
                          kind="ExternalInput").ap()
    bq = nc.dram_tensor("bq", [128, 2], f32, kind="ExternalInput").ap()
    mask128 = nc.dram_tensor("mask128", [128, 2, 128], f32,
                             kind="ExternalInput").ap()
    onesv = nc.dram_tensor("onesv", [128, LT, HPG, 1], val_dt,
                           kind="ExternalInput").ap()
    y = nc.dram_tensor("y", [L, D], f32, kind="ExternalOutput").ap()

    with tile.TileContext(nc) as tc:
        with tc.tile_pool(name="const", bufs=1) as cpool, \
             tc.tile_pool(name="qkvsb", bufs=1) as qpool, \
             tc.tile_pool(name="pt", bufs=3) as ptpool, \
             tc.tile_pool(name="ysb", bufs=2) as ypool, \
             tc.tile_pool(name="small", bufs=2) as spool, \
             tc.tile_pool(name="obp", bufs=2) as obpool:

            # ---- constants live for the whole kernel ----
            wout_t = [cpool.tile([128, D], pj_dt, tag=f"wout{k}",
                                 name=f"wout{k}") for k in range(2)]
            heat2 = cpool.tile([128, 512], mybir.dt.bfloat16, name="heat2")
            nc.vector.memset(heat2[:], 0.0)
            mask_t = cpool.tile([128, 2, 128], f32)

            # ---- persistent intermediates ----
            # Q^T / K^T: tile m holds heads (2m, 2m+1) of this group on
            # partitions 0-63 / 64-127. [128, L] each.
            qt_t = [qpool.tile([128, L], in_dt, tag=f"qt{m}", name=f"qt{m}")
                    for m in range(2)]
            kt_t = [qpool.tile([128, L], in_dt, tag=f"kt{m}", name=f"kt{m}")
                    for m in range(2)]
            # V (+ ones column): one tile, [128, LT, HPG, 65]
            vt = qpool.tile([128, LT, HPG, HD + 1], val_dt, name="vt")
            # attention out^T, same head layout as Q^T/K^T
            at_t = [qpool.tile([128, L], pj_dt, tag=f"at{m}", name=f"at{m}")
                    for m in range(2)]

            # ================= phase 1: QKV projections =================
            with tc.tile_pool(name="p1", bufs=1) as p1pool, \
                 tc.tile_pool(name="psqk", bufs=3, space="PSUM") as psqk, \
                 tc.tile_pool(name="psv", bufs=2, space="PSUM") as psv:
                wqk_t = p1pool.tile([128, KC, 2 * DG], in_dt, name="wqk_t")
                wv_t = p1pool.tile([128, KC, DG], in_dt, name="wv_t")
                bq_t = p1pool.tile([128, 2], f32)
                xt_t = p1pool.tile([128, KC, L], in_dt, name="xt_t")
                # big loads first, interleaved so the first matmuls' operands
                # arrive earliest; tiny scatter DMAs (ones) go last
                wqk_r = wqk.rearrange("k p n -> p k n")
                xT_r = xT.rearrange("k p n -> p k n")
                nc.sync.dma_start(wqk_t[:, 0:2, :], wqk_r[:, 0:2, :])
                nc.sync.dma_start(xt_t[:, 0:2, :], xT_r[:, 0:2, :])
                nc.sync.dma_start(wqk_t[:, 2:8, :], wqk_r[:, 2:8, :])
                nc.sync.dma_start(xt_t[:, 2:4, :], xT_r[:, 2:4, :])
                nc.sync.dma_start(xt_t[:, 4:6, :], xT_r[:, 4:6, :])
                nc.sync.dma_start(xt_t[:, 6:8, :], xT_r[:, 6:8, :])
                nc.sync.dma_start(wv_t[:], wv.rearrange("k p n -> p k n"))
                nc.sync.dma_start(bq_t[:], bq)
                nc.sync.dma_start(vt[:, :, :, HD:HD + 1], onesv)

                # PE heater: dependency-free matmuls that keep the PE array
                # busy while the input DMAs land, so the HAM clock-gate
                # un-throttles to 2.4 GHz before the real work starts.
                heat = p1pool.tile([128, 256], mybir.dt.bfloat16, name="heat")
                nc.vector.memset(heat[:], 0.0)
                psh_cm = tc.tile_pool(name="psheat", bufs=1, space="PSUM")
                psh = psh_cm.__enter__()
                hps = psh.tile([16, 256], f32, name="hps")
                for _ in range(120):
                    nc.tensor.matmul(hps[:], heat[:, 0:16], heat[:],
                                     start=True, stop=True)

                # Q^T and K^T: out[mc*128 rows of (q|k), 512 tokens]
                for mc in range(4):
                    dst = qt_t[mc] if mc < 2 else kt_t[mc - 2]
                    for nck in range(QC):
                        ps = psqk.tile([128, 512], f32, tag="psqk",
                                       name="psqk")
                        for k in range(KC):
                            nc.tensor.matmul(
                                ps[:],
                                wqk_t[:, k, bass.ts(mc, 128)],
                                xt_t[:, k, bass.ts(nck, 512)],
                                start=(k == 0), stop=(k == KC - 1),
                            )
                        if mc < 2:
                            nc.vector.tensor_scalar_add(
                                dst[:, bass.ts(nck, 512)], ps[:],
                                bq_t[:, mc:mc + 1])
                        else:
                            nc.vector.tensor_copy(
                                dst[:, bass.ts(nck, 512)], ps[:])
                        for _ in range(3):
                            nc.tensor.matmul(hps[:], heat[:, 0:16], heat[:],
                                             start=True, stop=True)
                # V: out[128 tokens, 256]
                for i in range(LT):
                    ps = psv.tile([128, DG], f32, tag="psv", name="psv")
                    for k in range(KC):
                        nc.tensor.matmul(
                            ps[:],
                            xt_t[:, k, bass.ts(i, 128)],
                            wv_t[:, k, :],
                            start=(k == 0), stop=(k == KC - 1),
                        )
                    nc.vector.tensor_copy(
                        vt[:, i, :, 0:HD],
                        ps[:].rearrange("p (h d) -> p h d", h=HPG),
                    )
                psh_cm.__exit__(None, None, None)

            # ====== phase 2+3: attention, with out-proj interleaved ======
            for k in range(2):
                nc.sync.dma_start(wout_t[k][:], wout[k])
            nc.sync.dma_start(mask_t[:], mask128)
            with tc.tile_pool(name="pss", bufs=2, space="PSUM") as pss, \
                 tc.tile_pool(name="pso", bufs=3, space="PSUM") as opool, \
                 tc.tile_pool(name="psh2", bufs=1, space="PSUM") as psh2:
                hps2 = psh2.tile([16, 512], f32, name="hps2")
                for qc in range(QC):
                    obs = {}
                    njt = 4 * qc + 4
                    for m in range(2):  # head pair (2m, 2m+1)
                        pso_p = [opool.tile([HD + 1, 512], f32, tag="o",
                                            name="o") for _ in range(2)]
                        for j in range(njt):
                            t = j - 4 * qc  # >=0 on diagonal k-tiles
                            c0 = 128 * t if t > 0 else 0
                            ps = pss.tile([128, 2, 512], f32, tag="pss",
                                          name="pss")
                            pt = ptpool.tile([128, 2, 512], val_dt, tag="pt",
                                             name="pt")
                            # the two heads' S^T land on disjoint PE row
                            # groups (partitions 0-63 / 64-127)
                            for e in range(2):
                                p0 = e * 64
                                nc.tensor.matmul(
                                    ps[:, e, c0:],
                                    kt_t[m][p0:p0 + 64, bass.ts(j, 128)],
                                    qt_t[m][p0:p0 + 64,
                                            512 * qc + c0:512 * (qc + 1)],
                                    start=True, stop=True,
                                )
                            if t >= 0:
                                nc.vector.tensor_add(
                                    ps[:, :, c0:c0 + 128],
                                    ps[:, :, c0:c0 + 128],
                                    mask_t[:])
                            nc.scalar.activation(
                                pt[:, :, c0:], ps[:, :, c0:],
                                mybir.ActivationFunctionType.Exp,
                                scale=1.0 / math.sqrt(HD),
                            )
                            nc.tensor.matmul(hps2[:], heat2[:, 0:16],
                                             heat2[:], start=True, stop=True)
                            for e in range(2):
                                nc.tensor.matmul(
                                    pso_p[e][:, c0:],
                                    vt[:, j, 2 * m + e, :],
                                    pt[:, e, c0:],
                                    start=(j == 0), stop=(j == njt - 1),
                                )
                        # copy out of PSUM immediately: frees the bank so
                        # the next chunk's PV can start; the slow reciprocal
                        # runs later, off the PSUM-recycle path
                        for e in range(2):
                            ob = obpool.tile([65, 512], f32, tag=f"ob{m}{e}",
                                             name="ob")
                            nc.vector.tensor_copy(ob[:], pso_p[e][:])
                            obs[(m, e)] = ob
                    for m in range(2):
                        for e in range(2):
                            p0 = e * 64
                            ob = obs[(m, e)]
                            rec = spool.tile([1, 512], f32, tag="rec",
                                             name="rec")
                            nc.vector.reciprocal(rec[:], ob[64:65, :])
                            rb = spool.tile([64, 512], f32, tag="rb",
                                            name="rb")
                            nc.gpsimd.partition_broadcast(rb[:], rec[:])
                            nc.vector.tensor_mul(
                                at_t[m][p0:p0 + 64, bass.ts(qc, 512)],
                                ob[0:64, :],
                                rb[:],
                            )
                    # out-proj lagged one qc so the normalize chains of
                    # this qc overlap the next qc's attention matmuls
                    prev = qc - 1
                    for i in ([] if prev < 0 else
                              range(4 * prev, 4 * prev + 4)):
                        yt = ypool.tile([128, D], f32, tag="yt", name="yt")
                        psy = [opool.tile([128, 512], f32, tag="o", name="o")
                               for _ in range(2)]
                        for k2 in range(2):
                            for n2 in range(2):
                                nc.tensor.matmul(
                                    psy[n2][:],
                                    at_t[k2][:, bass.ts(i, 128)],
                                    wout_t[k2][:, bass.ts(n2, 512)],
                                    start=(k2 == 0), stop=(k2 == 1),
                                )
                        nc.scalar.copy(yt[:, bass.ts(0, 512)], psy[0][:])
                        nc.vector.tensor_copy(yt[:, bass.ts(1, 512)],
                                              psy[1][:])
                        nc.sync.dma_start(y[bass.ts(i, 128), :], yt[:])
                for i in range(12, 16):
                    yt = ypool.tile([128, D], f32, tag="yt", name="yt")
                    psy = [opool.tile([128, 512], f32, tag="o", name="o")
                           for _ in range(2)]
                    for k2 in range(2):
                        for n2 in range(2):
                            nc.tensor.matmul(
                                psy[n2][:],
                                at_t[k2][:, bass.ts(i, 128)],
                                wout_t[k2][:, bass.ts(n2, 512)],
                                start=(k2 == 0), stop=(k2 == 1),
                            )
                    nc.scalar.copy(yt[:, bass.ts(0, 512)], psy[0][:])
                    nc.vector.tensor_copy(yt[:, bass.ts(1, 512)],
                                          psy[1][:])
                    nc.sync.dma_start(y[bass.ts(i, 128), :], yt[:])

    nc.compile()
    return nc


def _mask128_np():
    kk = np.arange(128)[:, None]
    qq = np.arange(128)[None, :]
    m1 = np.where(kk <= qq, 0.0, MASK_VAL).astype(np.float32)
    return np.ascontiguousarray(
        np.broadcast_to(m1[:, None, :], (128, 2, 128)))


def kernel(x, W_qkv, b_qkv, W_out, b_out):
    x = np.asarray(x, dtype=np.float32)
    W_qkv = np.asarray(W_qkv, dtype=np.float32)
    b_qkv = np.asarray(b_qkv, dtype=np.float32)
    W_out = np.asarray(W_out, dtype=np.float32)
    b_out = np.asarray(b_out, dtype=np.float32)

    if "nc" not in _CACHE:
        _CACHE["nc"] = _build()
    nc = _CACHE["nc"]

    Wq, Wk, Wv = W_qkv[:, :D], W_qkv[:, D:2 * D], W_qkv[:, 2 * D:]
    bq_full = b_qkv[:D]
    mask128 = _mask128_np()
    onesv = np.ones((128, LT, HPG, 1), dtype=np_val_dt)

    in_maps = []
    for c in range(NCORES):
        b, g = divmod(c, GROUPS)
        cs = slice(g * DG, (g + 1) * DG)
        xT_ = np.ascontiguousarray(x[b].T).astype(np_in_dt).reshape(
            KC, 128, L)
        wqk_ = np.ascontiguousarray(
            np.concatenate([Wq[:, cs], Wk[:, cs]], axis=1)
        ).astype(np_in_dt).reshape(KC, 128, 2 * DG)
        wv_ = np.ascontiguousarray(Wv[:, cs]).astype(np_in_dt).reshape(
            KC, 128, DG)
        wout_ = np.ascontiguousarray(W_out[cs, :]).astype(np_pj_dt).reshape(
            2, 128, D)
        bq_ = np.ascontiguousarray(bq_full[cs].reshape(2, 128).T)
        in_maps.append({
            "xT": xT_, "wqk": wqk_, "wv": wv_, "wout": wout_,
            "bq": bq_, "mask128": mask128, "onesv": onesv,
        })

    _CACHE["last_in_maps"] = in_maps
    res = run_bass_kernel_spmd(nc, in_maps, core_ids=list(range(NCORES)),
                               trace=False)
    _CACHE["last_results"] = res

    bias_row = b_out + b_qkv[2 * D:] @ W_out  # V-bias fold + output bias
    out = np.empty((B, L, D), dtype=np.float32)
    for b in range(B):
        acc = res.results[4 * b]["y"].astype(np.float64).copy()
        for g in range(1, GROUPS):
            acc += res.results[4 * b + g]["y"].astype(np.float64)
        out[b] = (acc + bias_row.astype(np.float64)).astype(np.float32)
    return out



# revision 5
# speedup vs baseline: 1.3290x; 1.3290x over previous
"""Causal self-attention (dense transformer) on 8 Trainium2 NeuronCores.

Problem: x[2, 2048, 1024], W_qkv[1024, 3072], b_qkv[3072], W_out[1024, 1024],
b_out[1024]; 16 heads, head_dim 64, causal softmax attention.

Sharding: 8 cores = 2 (batch) x 4 (head groups of 4 heads). Each core computes
QKV projection for its 4 heads, full causal attention for them, and a partial
output projection (its heads' rows of W_out). Host sums the 4 partials per
batch and adds the (bias) terms.

Device-side math notes:
  - K bias is dropped: adding a constant vector to every key shifts each
    query's scores by a per-query constant -> softmax invariant.
  - V bias is folded into the output bias on host: probs row-sums are 1, so
    attn = P @ (V + 1 c^T) = P@V + 1 c^T, and c^T @ W_out is a constant row.
  - Softmax has no max-subtraction: scores/8 have |.| < ~10 here, exp is safe.
  - Scores are computed transposed (S^T[k, q]) so no transposes are needed
    anywhere: softmax denominators come from a ones-column appended to V,
    and attention output lands directly in the [head_dim, token] layout the
    output projection needs as lhsT.
  - Strictly-above-diagonal k-tiles are never computed; the 128x128 blocks on
    the diagonal are masked POST-exp by a multiplicative 0/1 triangle (bf16,
    on the DVE), and the left-of-diagonal garbage columns inside a diagonal
    k-tile are simply never read by the PV matmul.
  - The whole datapath is bf16 (inputs, Q/K/V, probs, attention out, W_out);
    PSUM accumulation is fp32. rel-err budget is 2e-2; measured ~1e-3.
  - Softmax denominators: the ones-column of V gives the per-query sums as
    row 64 of the PV output.  A [1,512] single-lane reciprocal on the DVE
    costs 3.3us, so instead the row is DMA-reshaped to [128,4], reciprocal'd
    there (~0.1us), DMA'd back, then partition-broadcast (GPSIMD) and
    multiplied into the attention out tile.
  - Emission is software-pipelined: scores(j+1) is emitted BEFORE PV(j) so
    the in-order tensor queue never idles behind the exp(j) dependency; the
    lagged output projection is interleaved into the scores/PV stream to
    plug the remaining scalar-engine wait gaps.
  - Phase 1 (QKV) runs K^T with the contraction loop OUTERMOST across 8
    concurrently-open PSUM accumulation groups, so the PE array streams at
    DMA arrival pace instead of waiting for the full x load.
"""

import math

import ml_dtypes
import numpy as np

import concourse.bass as bass
import concourse.tile as tile
from concourse import bacc, mybir
from concourse.bass_utils import run_bass_kernel_spmd

B = 2
L = 2048
D = 1024
H = 16
HD = 64
NCORES = 8
GROUPS = 4  # head groups (tensor parallel)
HPG = H // GROUPS  # heads per group = 4
DG = HPG * HD  # 256 output dims per group
KC = D // 128  # 8 contraction chunks for QKV
LT = L // 128  # 16 token tiles
QC = L // 512  # 4 query chunks of 512
VW = HD + 2  # vt row stride (65 used, padded for alignment)

f32 = mybir.dt.float32
bf16 = mybir.dt.bfloat16
np_bf16 = np.dtype(ml_dtypes.bfloat16)

_CACHE = {}


def _build():
    nc = bacc.Bacc("TRN2", target_bir_lowering=False, debug=False,
                   num_devices=NCORES)

    xT = nc.dram_tensor("xT", [KC, 128, L], bf16, kind="ExternalInput").ap()
    wqk = nc.dram_tensor("wqk", [KC, 128, 2 * DG], bf16,
                         kind="ExternalInput").ap()
    wv = nc.dram_tensor("wv", [KC, 128, DG], bf16, kind="ExternalInput").ap()
    wout = nc.dram_tensor("wout", [2, 128, D], bf16,
                          kind="ExternalInput").ap()
    bq = nc.dram_tensor("bq", [128, 2], f32, kind="ExternalInput").ap()
    mask128 = nc.dram_tensor("mask128", [128, 2, 128], bf16,
                             kind="ExternalInput").ap()
    y = nc.dram_tensor("y", [L, D], f32, kind="ExternalOutput").ap()

    with tile.TileContext(nc) as tc:
        with tc.tile_pool(name="const", bufs=1) as cpool, \
             tc.tile_pool(name="qkvsb", bufs=1) as qpool, \
             tc.tile_pool(name="pt", bufs=3) as ptpool, \
             tc.tile_pool(name="ysb", bufs=2) as ypool, \
             tc.tile_pool(name="small", bufs=2) as spool, \
             tc.tile_pool(name="obp", bufs=2) as obpool:

            # ---- constants live for the whole kernel ----
            wout_t = [cpool.tile([128, D], bf16, tag=f"wout{k}",
                                 name=f"wout{k}") for k in range(2)]
            mask_t = cpool.tile([128, 2, 128], bf16)

            # ---- persistent intermediates ----
            # Q^T / K^T: tile m holds heads (2m, 2m+1) of this group on
            # partitions 0-63 / 64-127. [128, L] each.
            qt_t = [qpool.tile([128, L], bf16, tag=f"qt{m}", name=f"qt{m}")
                    for m in range(2)]
            kt_t = [qpool.tile([128, L], bf16, tag=f"kt{m}", name=f"kt{m}")
                    for m in range(2)]
            # V (+ ones column): one tile, [128, LT, HPG, VW]
            vt = qpool.tile([128, LT, HPG, VW], bf16, name="vt")
            # attention out^T, same head layout as Q^T/K^T
            at_t = [qpool.tile([128, L], bf16, tag=f"at{m}", name=f"at{m}")
                    for m in range(2)]

            # ================= phase 1: QKV projections =================
            # All phase-1 PSUM comes from ONE 8-slot ring of [128,512] bank
            # tiles (16KB/partition = the whole PSUM).
            with tc.tile_pool(name="p1", bufs=1) as p1pool, \
                 tc.tile_pool(name="psa", bufs=8, space="PSUM") as psa:
                wqk_t = p1pool.tile([128, KC, 2 * DG], bf16, name="wqk_t")
                wv_t = p1pool.tile([128, KC, DG], bf16, name="wv_t")
                bq_t = p1pool.tile([128, 2], f32)
                xt_t = p1pool.tile([128, KC, L], bf16, name="xt_t")
                # chunked loads, contraction-ordered, so the k-outer matmul
                # stream below can chase the DMA arrivals
                wqk_r = wqk.rearrange("k p n -> p k n")
                xT_r = xT.rearrange("k p n -> p k n")
                wv_r = wv.rearrange("k p n -> p k n")
                for k in range(KC):
                    nc.sync.dma_start(wqk_t[:, k, :], wqk_r[:, k, :])
                    nc.sync.dma_start(xt_t[:, k, :], xT_r[:, k, :])
                    nc.sync.dma_start(wv_t[:, k, :], wv_r[:, k, :])
                nc.sync.dma_start(bq_t[:], bq)
                nc.vector.memset(vt[:, :, :, HD:HD + 1], 1.0)

                # PE heater: dependency-free matmuls to lift the HAM clock
                # gate while the first input chunks land.
                heat = p1pool.tile([128, 256], bf16, name="heat")
                nc.vector.memset(heat[:], 0.0)
                hps = psa.tile([16, 256], f32, tag="psa", name="hps")
                for _ in range(8):
                    nc.tensor.matmul(hps[:], heat[:, 0:16], heat[:],
                                     start=True, stop=True)

                # K^T: 8 accumulation groups open at once (8 PSUM banks),
                # contraction loop outermost -> streams at DMA pace.
                kt_ps = [psa.tile([128, 512], f32, tag="psa",
                                  name=f"ktps{g}") for g in range(8)]
                for k in range(KC):
                    for mi, mc in enumerate((2, 3)):
                        for nck in range(QC):
                            nc.tensor.matmul(
                                kt_ps[4 * mi + nck][:],
                                wqk_t[:, k, bass.ts(mc, 128)],
                                xt_t[:, k, bass.ts(nck, 512)],
                                start=(k == 0), stop=(k == KC - 1),
                            )
                for mi in range(2):
                    for nck in range(QC):
                        nc.vector.tensor_copy(
                            kt_t[mi][:, bass.ts(nck, 512)],
                            kt_ps[4 * mi + nck][:])

                # V: k-inner (x is resident by now); two groups share a
                # [128,512] PSUM tile in halves.
                for ih in range(LT // 2):
                    psv = psa.tile([128, 512], f32, tag="psa", name="psv")
                    for half in range(2):
                        i = 2 * ih + half
                        for k in range(KC):
                            nc.tensor.matmul(
                                psv[:, bass.ts(half, DG)],
                                xt_t[:, k, bass.ts(i, 128)],
                                wv_t[:, k, :],
                                start=(k == 0), stop=(k == KC - 1),
                            )
                    for half in range(2):
                        i = 2 * ih + half
                        nc.vector.tensor_copy(
                            vt[:, i, :, 0:HD],
                            psv[:, bass.ts(half, DG)].rearrange(
                                "p (h d) -> p h d", h=HPG),
                        )

                # Q^T (+ bias): k-inner, full rate.
                for mc in range(2):
                    for nck in range(QC):
                        ps = psa.tile([128, 512], f32, tag="psa", name="psq")
                        for k in range(KC):
                            nc.tensor.matmul(
                                ps[:],
                                wqk_t[:, k, bass.ts(mc, 128)],
                                xt_t[:, k, bass.ts(nck, 512)],
                                start=(k == 0), stop=(k == KC - 1),
                            )
                        nc.vector.tensor_scalar_add(
                            qt_t[mc][:, bass.ts(nck, 512)], ps[:],
                            bq_t[:, mc:mc + 1])

            # ====== phase 2+3: attention, with out-proj interleaved ======
            for k in range(2):
                nc.sync.dma_start(wout_t[k][:], wout[k])
            nc.sync.dma_start(mask_t[:], mask128)

            def emit_outproj(i):
                """One token tile of the (lagged) output projection."""
                yt = ypool.tile([128, D], f32, tag="yt", name="yt")
                psy = pss.tile([128, 2, 512], f32, tag="pss", name="pssy")
                for k2 in range(2):
                    for n2 in range(2):
                        nc.tensor.matmul(
                            psy[:, n2, :],
                            at_t[k2][:, bass.ts(i, 128)],
                            wout_t[k2][:, bass.ts(n2, 512)],
                            start=(k2 == 0), stop=(k2 == 1),
                        )
                nc.vector.tensor_copy(yt[:],
                                      psy[:].rearrange("p a b -> p (a b)"))
                nc.sync.dma_start(y[bass.ts(i, 128), :], yt[:])

            with tc.tile_pool(name="pss", bufs=2, space="PSUM") as pss, \
                 tc.tile_pool(name="pso", bufs=2, space="PSUM") as opool:
                deferred = []  # queued DVE/GPSIMD normalize work

                def emit_scores(m, qc, j, njt):
                    t = j - 4 * qc
                    c0 = 128 * t if t > 0 else 0
                    ps = pss.tile([128, 2, 512], f32, tag="pss", name="pss")
                    for e in range(2):
                        p0 = e * 64
                        nc.tensor.matmul(
                            ps[:, e, c0:],
                            kt_t[m][p0:p0 + 64, bass.ts(j, 128)],
                            qt_t[m][p0:p0 + 64,
                                    512 * qc + c0:512 * (qc + 1)],
                            start=True, stop=True,
                        )
                    return ps

                def emit_exp_pv(m, qc, j, njt, ps, pso_t):
                    t = j - 4 * qc
                    c0 = 128 * t if t > 0 else 0
                    pt = ptpool.tile([128, 2, 512], bf16, tag="pt",
                                     name="pt")
                    nc.scalar.activation(
                        pt[:, :, c0:], ps[:, :, c0:],
                        mybir.ActivationFunctionType.Exp,
                        scale=1.0 / math.sqrt(HD),
                    )
                    if t >= 0:
                        # post-exp multiplicative causal mask (0/1 triangle)
                        nc.vector.tensor_mul(
                            pt[:, :, c0:c0 + 128],
                            pt[:, :, c0:c0 + 128],
                            mask_t[:])
                    for e in range(2):
                        nc.tensor.matmul(
                            pso_t[:, e, c0:],
                            vt[:, j, 2 * m + e, 0:HD + 1],
                            pt[:, e, c0:],
                            start=(j == 0), stop=(j == njt - 1),
                        )

                for qc in range(QC):
                    njt = 4 * qc + 4
                    # out-proj token tiles of the previous chunk, interleaved
                    # into the scores/PV stream to fill PE wait gaps
                    opq = list(range(4 * (qc - 1), 4 * qc)) if qc else []
                    for m in range(2):  # head pair (2m, 2m+1)
                        # flush pending normalize chains (their DMA round
                        # trips have had a full m-loop of slack by now)
                        for fn in deferred:
                            fn()
                        deferred.clear()
                        pso_t = opool.tile([HD + 1, 2, 512], f32, tag="o",
                                           name="o")
                        ps_prev = emit_scores(m, qc, 0, njt)
                        for j in range(njt):
                            # pipeline: scores(j+1) issued before PV(j) so
                            # the tensor queue never blocks behind exp(j)
                            ps_next = (emit_scores(m, qc, j + 1, njt)
                                       if j + 1 < njt else None)
                            emit_exp_pv(m, qc, j, njt, ps_prev, pso_t)
                            ps_prev = ps_next
                            if j % 4 == 1 and opq:
                                emit_outproj(opq.pop(0))
                        obs = []
                        for e in range(2):
                            ob = obpool.tile([HD + 1, 512], f32,
                                             tag=f"ob{m}{e}", name="ob")
                            nc.vector.tensor_copy(ob[:], pso_t[:, e, :])
                            obs.append(ob)
                        # denominator row -> [128,4] via DMA for a batched
                        # reciprocal; the DVE/GPSIMD consumers are deferred
                        # so the in-order queues don't stall on the DMA.
                        for e in range(2):
                            ob = obs[e]
                            dn = spool.tile([128, 4], f32, tag="dn",
                                            name="dn")
                            nc.sync.dma_start(dn[:], ob[HD:HD + 1, :])
                            deferred.append(
                                lambda m=m, e=e, qc=qc, ob=ob, dn=dn:
                                _emit_normalize(nc, spool, at_t, m, e, qc,
                                                ob, dn))
                for fn in deferred:
                    fn()
                for i in range(12, 16):
                    emit_outproj(i)

    nc.compile()
    return nc


def _emit_normalize(nc, spool, at_t, m, e, qc, ob, dn):
    p0 = e * 64
    rc = spool.tile([128, 4], f32, tag="rc", name="rc")
    nc.vector.reciprocal(rc[:], dn[:])
    rrow = spool.tile([1, 512], f32, tag="rrow", name="rrow")
    nc.sync.dma_start(rrow[:], rc[:])
    rb = spool.tile([64, 512], f32, tag="rb", name="rb")
    nc.gpsimd.partition_broadcast(rb[:], rrow[:])
    nc.vector.tensor_mul(
        at_t[m][p0:p0 + 64, bass.ts(qc, 512)],
        ob[0:HD, :],
        rb[:],
    )


def _mask128_np():
    kk = np.arange(128)[:, None]
    qq = np.arange(128)[None, :]
    m1 = np.where(kk <= qq, 1.0, 0.0).astype(np_bf16)
    return np.ascontiguousarray(
        np.broadcast_to(m1[:, None, :], (128, 2, 128)))


def kernel(x, W_qkv, b_qkv, W_out, b_out):
    x = np.asarray(x, dtype=np.float32)
    W_qkv = np.asarray(W_qkv, dtype=np.float32)
    b_qkv = np.asarray(b_qkv, dtype=np.float32)
    W_out = np.asarray(W_out, dtype=np.float32)
    b_out = np.asarray(b_out, dtype=np.float32)

    if "nc" not in _CACHE:
        _CACHE["nc"] = _build()
    nc = _CACHE["nc"]

    Wq, Wk, Wv = W_qkv[:, :D], W_qkv[:, D:2 * D], W_qkv[:, 2 * D:]
    bq_full = b_qkv[:D]
    mask128 = _mask128_np()

    in_maps = []
    for c in range(NCORES):
        b, g = divmod(c, GROUPS)
        cs = slice(g * DG, (g + 1) * DG)
        xT_ = np.ascontiguousarray(x[b].T).astype(np_bf16).reshape(
            KC, 128, L)
        wqk_ = np.ascontiguousarray(
            np.concatenate([Wq[:, cs], Wk[:, cs]], axis=1)
        ).astype(np_bf16).reshape(KC, 128, 2 * DG)
        wv_ = np.ascontiguousarray(Wv[:, cs]).astype(np_bf16).reshape(
            KC, 128, DG)
        wout_ = np.ascontiguousarray(W_out[cs, :]).astype(np_bf16).reshape(
            2, 128, D)
        bq_ = np.ascontiguousarray(bq_full[cs].reshape(2, 128).T)
        in_maps.append({
            "xT": xT_, "wqk": wqk_, "wv": wv_, "wout": wout_,
            "bq": bq_, "mask128": mask128,
        })

    _CACHE["last_in_maps"] = in_maps
    res = run_bass_kernel_spmd(nc, in_maps, core_ids=list(range(NCORES)),
                               trace=False)
    _CACHE["last_results"] = res

    bias_row = b_out + b_qkv[2 * D:] @ W_out  # V-bias fold + output bias
    out = np.empty((B, L, D), dtype=np.float32)
    for b in range(B):
        acc = res.results[4 * b]["y"].astype(np.float64).copy()
        for g in range(1, GROUPS):
            acc += res.results[4 * b + g]["y"].astype(np.float64)
        out[b] = (acc + bias_row.astype(np.float64)).astype(np.float32)
    return out


# revision 9
# speedup vs baseline: 1.6048x; 1.2076x over previous
"""Causal self-attention (dense transformer) on 8 Trainium2 NeuronCores.

Problem: x[2, 2048, 1024], W_qkv[1024, 3072], b_qkv[3072], W_out[1024, 1024],
b_out[1024]; 16 heads, head_dim 64, causal softmax attention.

Sharding: 8 cores = 2 (batch) x 4 (head groups of 4 heads). Each core computes
QKV projection for its 4 heads, full causal attention for them, and a partial
output projection (its heads' rows of W_out). Host sums the 4 partials per
batch and adds the (bias) terms.

Device-side math notes:
  - K bias is dropped: adding a constant vector to every key shifts each
    query's scores by a per-query constant -> softmax invariant.
  - V bias is folded into the output bias on host: probs row-sums are 1, so
    attn = P @ (V + 1 c^T) = P@V + 1 c^T, and c^T @ W_out is a constant row.
  - Softmax has no max-subtraction: scores/8 have |.| < ~10 here, exp is safe.
  - Scores are computed transposed (S^T[k, q]) so no transposes are needed
    anywhere: softmax denominators come from a ones-column appended to V,
    and attention output lands directly in the [head_dim, token] layout the
    output projection needs as lhsT.
  - Strictly-above-diagonal k-tiles are never computed; the 128x128 blocks on
    the diagonal are masked POST-exp by a multiplicative 0/1 triangle (bf16,
    on the DVE), and the left-of-diagonal garbage columns inside a diagonal
    k-tile are simply never read by the PV matmul.
  - The whole datapath is bf16 (inputs, Q/K/V, probs, attention out, W_out);
    PSUM accumulation is fp32. rel-err budget is 2e-2; measured ~3e-3.
  - Softmax denominators: the ones-column of V gives the per-query sums as
    row 64 of the PV output.  A [1,512] single-lane reciprocal on the DVE
    costs 3.3us, so instead the row is DMA-reshaped to [128,4], reciprocal'd
    there (~0.1us), DMA'd back, then partition-broadcast (GPSIMD) and
    multiplied into the attention out tile.  These chains are emitted
    deferred (at the next head-pair group start) so the in-order DVE queue
    never stalls on the DMA round trip.
  - The attention stream is a single software-pipelined sequence over all
    (q-chunk, head-pair, k-tile) steps: scores(step i+1) is emitted BEFORE
    PV(step i), so the in-order tensor queue never blocks behind exp(i),
    including across group boundaries.
  - Only K^T, V[0..3] and Q^T[qc0] are computed up front; the remaining V
    token-tiles, Q^T chunks and the (lagged) output projection are spliced
    one accumulation group at a time into the attention stream as filler
    work that keeps the PE busy while the scalar engine works through the
    exp backlog.  K^T runs with the contraction loop outermost across 8
    concurrently-open PSUM banks so the PE streams at DMA arrival pace.
  - The exp activation table is preloaded with a dummy activation at t=0 so
    the first real exp doesn't eat the ~2.7us ACT_TABLE_LOAD stall.
"""

import math

import ml_dtypes
import numpy as np

import concourse.bass as bass
import concourse.tile as tile
from concourse import bacc, mybir
from concourse.bass_utils import run_bass_kernel_spmd

B = 2
L = 2048
D = 1024
H = 16
HD = 64
NCORES = 8
GROUPS = 4  # head groups (tensor parallel)
HPG = H // GROUPS  # heads per group = 4
DG = HPG * HD  # 256 output dims per group
KC = D // 128  # 8 contraction chunks for QKV
LT = L // 128  # 16 token tiles
QC = L // 512  # 4 query chunks of 512
VW = HD + 2  # vt row stride (65 used, padded for alignment)

f32 = mybir.dt.float32
bf16 = mybir.dt.bfloat16
np_bf16 = np.dtype(ml_dtypes.bfloat16)

_CACHE = {}


def _build():
    nc = bacc.Bacc("TRN2", target_bir_lowering=False, debug=False,
                   num_devices=NCORES)

    xT = nc.dram_tensor("xT", [KC, 128, L], bf16, kind="ExternalInput").ap()
    wqk = nc.dram_tensor("wqk", [KC, 128, 2 * DG], bf16,
                         kind="ExternalInput").ap()
    wv = nc.dram_tensor("wv", [KC, 128, DG], bf16, kind="ExternalInput").ap()
    wout = nc.dram_tensor("wout", [2, 128, D], bf16,
                          kind="ExternalInput").ap()
    bq = nc.dram_tensor("bq", [128, 2], f32, kind="ExternalInput").ap()
    mask128 = nc.dram_tensor("mask128", [128, 2, 128], bf16,
                             kind="ExternalInput").ap()
    y = nc.dram_tensor("y", [L, D], f32, kind="ExternalOutput").ap()

    with tile.TileContext(nc) as tc:
        with tc.tile_pool(name="const", bufs=1) as cpool, \
             tc.tile_pool(name="qkvsb", bufs=1) as qpool, \
             tc.tile_pool(name="pt", bufs=3) as ptpool, \
             tc.tile_pool(name="ysb", bufs=2) as ypool, \
             tc.tile_pool(name="small", bufs=2) as spool, \
             tc.tile_pool(name="obp", bufs=2) as obpool:

            # ---- constants live for the whole kernel ----
            wout_t = [cpool.tile([128, D], bf16, tag=f"wout{k}",
                                 name=f"wout{k}") for k in range(2)]
            mask_t = cpool.tile([128, 2, 128], bf16)
            # phase-1 operands stay resident: V/Q^T filler groups inside the
            # attention stream keep reading them
            wqk_t = cpool.tile([128, KC, 2 * DG], bf16, name="wqk_t")
            wv_t = cpool.tile([128, KC, DG], bf16, name="wv_t")
            bq_t = cpool.tile([128, 2], f32)
            xt_t = cpool.tile([128, KC, L], bf16, name="xt_t")

            # ---- persistent intermediates ----
            # Q^T / K^T: tile m holds heads (2m, 2m+1) of this group on
            # partitions 0-63 / 64-127. [128, L] each.
            qt_t = [qpool.tile([128, L], bf16, tag=f"qt{m}", name=f"qt{m}")
                    for m in range(2)]
            kt_t = [qpool.tile([128, L], bf16, tag=f"kt{m}", name=f"kt{m}")
                    for m in range(2)]
            # V (+ ones column): one tile, [128, LT, HPG, VW]
            vt = qpool.tile([128, LT, HPG, VW], bf16, name="vt")
            # attention out^T, same head layout as Q^T/K^T
            at_t = [qpool.tile([128, L], bf16, tag=f"at{m}", name=f"at{m}")
                    for m in range(2)]

            # chunked loads, contraction-ordered, so the k-outer matmul
            # stream below can chase the DMA arrivals
            wqk_r = wqk.rearrange("k p n -> p k n")
            xT_r = xT.rearrange("k p n -> p k n")
            wv_r = wv.rearrange("k p n -> p k n")
            for k in range(KC):
                nc.sync.dma_start(wqk_t[:, k, :], wqk_r[:, k, :])
                nc.sync.dma_start(xt_t[:, k, :], xT_r[:, k, :])
                nc.sync.dma_start(wv_t[:, k, :], wv_r[:, k, :])
            nc.sync.dma_start(bq_t[:], bq)
            nc.vector.memset(vt[:, :, :, HD:HD + 1], 1.0)

            heat = cpool.tile([128, 256], bf16, name="heat")
            nc.vector.memset(heat[:], 0.0)
            # preload the exp activation table while the inputs stream in
            dume = spool.tile([128, 1], f32, tag="dume", name="dume")
            nc.scalar.activation(dume[:], heat[:, 0:1],
                                 mybir.ActivationFunctionType.Exp)

            # ---------- V / Q^T accumulation-group emitters ----------
            def emit_v_group(i, pool):
                psv = pool.tile([128, 512], f32, tag=pool_tag(pool),
                                name="psv")
                for k in range(KC):
                    nc.tensor.matmul(
                        psv[:, 0:DG],
                        xt_t[:, k, bass.ts(i, 128)],
                        wv_t[:, k, :],
                        start=(k == 0), stop=(k == KC - 1),
                    )
                nc.vector.tensor_copy(
                    vt[:, i, :, 0:HD],
                    psv[:, 0:DG].rearrange("p (h d) -> p h d", h=HPG),
                )

            def emit_q_group(mc, nck, pool):
                psq = pool.tile([128, 512], f32, tag=pool_tag(pool),
                                name="psq")
                for k in range(KC):
                    nc.tensor.matmul(
                        psq[:],
                        wqk_t[:, k, bass.ts(mc, 128)],
                        xt_t[:, k, bass.ts(nck, 512)],
                        start=(k == 0), stop=(k == KC - 1),
                    )
                nc.vector.tensor_scalar_add(
                    qt_t[mc][:, bass.ts(nck, 512)], psq[:],
                    bq_t[:, mc:mc + 1])

            def pool_tag(pool):
                return "psa" if pool is not None and pool.name == "psa" \
                    else "psf"

            # ================= phase 1: K^T, V[0:4], Q^T[0] =================
            with tc.tile_pool(name="psa", bufs=8, space="PSUM") as psa:
                # PE heater: dependency-free matmuls to lift the HAM clock
                # gate while the first input chunks land.
                hps = psa.tile([16, 256], f32, tag="psa", name="hps")
                for _ in range(24):
                    nc.tensor.matmul(hps[:], heat[:, 0:16], heat[:],
                                     start=True, stop=True)

                # K^T: 8 accumulation groups open at once (8 PSUM banks),
                # contraction loop outermost -> streams at DMA pace.
                kt_ps = [psa.tile([128, 512], f32, tag="psa",
                                  name=f"ktps{g}") for g in range(8)]
                for k in range(KC):
                    for mi, mc in enumerate((2, 3)):
                        for nck in range(QC):
                            nc.tensor.matmul(
                                kt_ps[4 * mi + nck][:],
                                wqk_t[:, k, bass.ts(mc, 128)],
                                xt_t[:, k, bass.ts(nck, 512)],
                                start=(k == 0), stop=(k == KC - 1),
                            )
                for mi in range(2):
                    for nck in range(QC):
                        nc.vector.tensor_copy(
                            kt_t[mi][:, bass.ts(nck, 512)],
                            kt_ps[4 * mi + nck][:])

                for i in range(4):
                    emit_v_group(i, psa)
                for mc in range(2):
                    emit_q_group(mc, 0, psa)

            # ====== phase 2: attention; V/Q^T/out-proj spliced in ======
            for k in range(2):
                nc.sync.dma_start(wout_t[k][:], wout[k])
            nc.sync.dma_start(mask_t[:], mask128)

            with tc.tile_pool(name="pss", bufs=2, space="PSUM") as pss, \
                 tc.tile_pool(name="pso", bufs=1, space="PSUM") as opool, \
                 tc.tile_pool(name="psf", bufs=2, space="PSUM") as psf:
                deferred = []   # pending normalize chains
                pso_cur = [None]

                def emit_outproj(i):
                    """One token tile of the (lagged) output projection."""
                    yt = ypool.tile([128, D], f32, tag="yt", name="yt")
                    psy = [psf.tile([128, 512], f32, tag="psf",
                                    name="psy") for _ in range(2)]
                    for k2 in range(2):
                        for n2 in range(2):
                            nc.tensor.matmul(
                                psy[n2][:],
                                at_t[k2][:, bass.ts(i, 128)],
                                wout_t[k2][:, bass.ts(n2, 512)],
                                start=(k2 == 0), stop=(k2 == 1),
                            )
                    for n2 in range(2):
                        nc.vector.tensor_copy(yt[:, bass.ts(n2, 512)],
                                              psy[n2][:])
                    nc.sync.dma_start(y[bass.ts(i, 128), :], yt[:])

                def emit_scores(m, qc, j):
                    t = j - 4 * qc
                    c0 = 128 * t if t > 0 else 0
                    ps = pss.tile([128, 2, 512], f32, tag="pss", name="pss")
                    for e in range(2):
                        p0 = e * 64
                        nc.tensor.matmul(
                            ps[:, e, c0:],
                            kt_t[m][p0:p0 + 64, bass.ts(j, 128)],
                            qt_t[m][p0:p0 + 64,
                                    512 * qc + c0:512 * (qc + 1)],
                            start=True, stop=True,
                        )
                    return ps

                def emit_exp_pv(m, qc, j, njt, ps):
                    t = j - 4 * qc
                    c0 = 128 * t if t > 0 else 0
                    if j == 0:
                        pso_cur[0] = opool.tile([HD + 1, 2, 512], f32,
                                                tag="o", name="o")
                    pso_t = pso_cur[0]
                    pt = ptpool.tile([128, 2, 512], bf16, tag="pt",
                                     name="pt")
                    nc.scalar.activation(
                        pt[:, :, c0:], ps[:, :, c0:],
                        mybir.ActivationFunctionType.Exp,
                        scale=1.0 / math.sqrt(HD),
                    )
                    if t >= 0:
                        # post-exp multiplicative causal mask (0/1 triangle)
                        nc.vector.tensor_mul(
                            pt[:, :, c0:c0 + 128],
                            pt[:, :, c0:c0 + 128],
                            mask_t[:])
                    for e in range(2):
                        nc.tensor.matmul(
                            pso_t[:, e, c0:],
                            vt[:, j, 2 * m + e, 0:HD + 1],
                            pt[:, e, c0:],
                            start=(j == 0), stop=(j == njt - 1),
                        )
                    if j == njt - 1:
                        for e in range(2):
                            ob = obpool.tile([HD + 1, 512], f32,
                                             tag=f"ob{m}{e}", name="ob")
                            nc.vector.tensor_copy(ob[:], pso_t[:, e, :])
                            # denominator row -> [128,4] via DMA for a
                            # batched reciprocal; consumers are deferred so
                            # the in-order queues don't stall on the DMA
                            dn = spool.tile([128, 4], f32, tag="dn",
                                            name="dn", bufs=4)
                            nc.sync.dma_start(dn[:], ob[HD:HD + 1, :])
                            deferred.append((
                                2 * qc + m,
                                lambda m=m, e=e, qc=qc, ob=ob, dn=dn:
                                _emit_normalize(nc, spool, at_t, m, e, qc,
                                                ob, dn)))

                # flat step list over (qc, m, j); filler work (V / Q^T /
                # out-proj accumulation groups) is scheduled per head-pair
                # GROUP so every filler lands after its inputs are emitted:
                #  - V[4i..] needed by PV two groups later
                #  - qt[qc+1] needed by the lookahead scores at the END of
                #    group 2*qc+1, so emit in that group's early steps
                #  - out-proj of qc's tokens needs normalize(qc, m0+m1),
                #    which flush at group starts 2*qc+2 / 2*qc+3
                steps = []
                for qc in range(QC):
                    for m in range(2):
                        for j in range(4 * qc + 4):
                            steps.append((qc, m, j))
                fillers = {
                    0: [lambda i=i: emit_v_group(i, psf)
                        for i in range(4, 8)],
                    1: [lambda mc=mc: emit_q_group(mc, 1, psf)
                        for mc in range(2)],
                    2: [lambda i=i: emit_v_group(i, psf)
                        for i in range(8, 12)],
                    3: [lambda mc=mc: emit_q_group(mc, 2, psf)
                        for mc in range(2)],
                    4: [lambda i=i: emit_v_group(i, psf)
                        for i in range(12, 16)],
                    5: [lambda mc=mc: emit_q_group(mc, 3, psf)
                        for mc in range(2)] +
                       [lambda i=i: emit_outproj(i) for i in range(0, 2)],
                    6: [lambda i=i: emit_outproj(i) for i in range(2, 8)],
                    7: [lambda i=i: emit_outproj(i) for i in range(8, 12)],
                }
                filler_q = []
                cur_g = -1
                ps_next = emit_scores(steps[0][1], steps[0][0], steps[0][2])
                for si, (qc, m, j) in enumerate(steps):
                    g = 2 * qc + m
                    if g != cur_g:
                        cur_g = g
                        # flush normalize chains two groups behind: their
                        # DMA round trips have had a full group of slack
                        while deferred and deferred[0][0] <= g - 2:
                            deferred.pop(0)[1]()
                        filler_q.extend(fillers.get(g, []))
                    ps_cur = ps_next
                    if si + 1 < len(steps):
                        nqc, nm, nj = steps[si + 1]
                        ps_next = emit_scores(nm, nqc, nj)
                    emit_exp_pv(m, qc, j, 4 * qc + 4, ps_cur)
                    if filler_q:
                        filler_q.pop(0)()
                for _, fn in deferred:
                    fn()
                deferred.clear()
                for i in range(12, 16):
                    emit_outproj(i)

    nc.compile()
    return nc


def _emit_normalize(nc, spool, at_t, m, e, qc, ob, dn):
    p0 = e * 64
    rc = spool.tile([128, 4], f32, tag="rc", name="rc")
    nc.vector.reciprocal(rc[:], dn[:])
    rrow = spool.tile([1, 512], f32, tag="rrow", name="rrow")
    nc.sync.dma_start(rrow[:], rc[:])
    rb = spool.tile([64, 512], f32, tag="rb", name="rb")
    nc.gpsimd.partition_broadcast(rb[:], rrow[:])
    nc.vector.tensor_mul(
        at_t[m][p0:p0 + 64, bass.ts(qc, 512)],
        ob[0:HD, :],
        rb[:],
    )


def _mask128_np():
    kk = np.arange(128)[:, None]
    qq = np.arange(128)[None, :]
    m1 = np.where(kk <= qq, 1.0, 0.0).astype(np_bf16)
    return np.ascontiguousarray(
        np.broadcast_to(m1[:, None, :], (128, 2, 128)))


def kernel(x, W_qkv, b_qkv, W_out, b_out):
    x = np.asarray(x, dtype=np.float32)
    W_qkv = np.asarray(W_qkv, dtype=np.float32)
    b_qkv = np.asarray(b_qkv, dtype=np.float32)
    W_out = np.asarray(W_out, dtype=np.float32)
    b_out = np.asarray(b_out, dtype=np.float32)

    if "nc" not in _CACHE:
        _CACHE["nc"] = _build()
    nc = _CACHE["nc"]

    Wq, Wk, Wv = W_qkv[:, :D], W_qkv[:, D:2 * D], W_qkv[:, 2 * D:]
    bq_full = b_qkv[:D]
    mask128 = _mask128_np()

    in_maps = []
    for c in range(NCORES):
        b, g = divmod(c, GROUPS)
        cs = slice(g * DG, (g + 1) * DG)
        xT_ = np.ascontiguousarray(x[b].T).astype(np_bf16).reshape(
            KC, 128, L)
        wqk_ = np.ascontiguousarray(
            np.concatenate([Wq[:, cs], Wk[:, cs]], axis=1)
        ).astype(np_bf16).reshape(KC, 128, 2 * DG)
        wv_ = np.ascontiguousarray(Wv[:, cs]).astype(np_bf16).reshape(
            KC, 128, DG)
        wout_ = np.ascontiguousarray(W_out[cs, :]).astype(np_bf16).reshape(
            2, 128, D)
        bq_ = np.ascontiguousarray(bq_full[cs].reshape(2, 128).T)
        in_maps.append({
            "xT": xT_, "wqk": wqk_, "wv": wv_, "wout": wout_,
            "bq": bq_, "mask128": mask128,
        })

    _CACHE["last_in_maps"] = in_maps
    res = run_bass_kernel_spmd(nc, in_maps, core_ids=list(range(NCORES)),
                               trace=False)
    _CACHE["last_results"] = res

    bias_row = b_out + b_qkv[2 * D:] @ W_out  # V-bias fold + output bias
    out = np.empty((B, L, D), dtype=np.float32)
    for b in range(B):
        acc = res.results[4 * b]["y"].astype(np.float64).copy()
        for g in range(1, GROUPS):
            acc += res.results[4 * b + g]["y"].astype(np.float64)
        out[b] = (acc + bias_row.astype(np.float64)).astype(np.float32)
    return out
